# revision 15
# baseline (speedup 1.0000x reference)
"""Trainium2 Bass kernel for LowRankBilinearAttention.

Reference computation (per batch b):
    a1 = x1 @ W1                  # (P, ATT)
    a2 = x2 @ W2                  # (L, ATT)
    u  = tanh(a1[p,:] * a2[l,:])  # (L, P, ATT)
    w  = Wh @ Wt                  # (ATT,)   [folded on host]
    score[l,p] = u . w            # + (bh@Wt + bt) -- a scalar, cancels in softmax
    alpha = softmax(score, axis=p)
    label = alpha @ x1            # (L, DIM1)

Sharding: pure data-parallel over batch B=8 -> one batch per NeuronCore,
no collectives.

Device layout (per core), fp16 datapath with fp32 accumulation:
    ATT on partitions (8 blocks of 128) for the u stage.  Per group of
    GR=8 l-values:
      - DVE/GpSimd tensor_tensor with stride-0 broadcast APs:
        up[a, k, l, p] = a1T[a,p] * a2T[a,l]      (fp16, one instr per k)
      - ACT tanh in-place over the whole (128, 12544) chunk
      - PE: M=1 fp16 matmuls (lhsT = w column) with N=392 (two l per psum
        row), accumulating over k into one PSUM bank at rows {0,32,64,96}
        (legal output base partitions via tile_position).
    Softmax is per-partition per-slot (ACT exp with per-partition bias and
    accum_out), so the scattered row layout is fine; alpha rows are gathered
    by partition-strided DMA; alpha is normalized before the PE transpose so
    the label matmul needs no rescale.
"""

import sys

import numpy as np

if "/opt/trn_rl_repo" not in sys.path:
    sys.path.insert(0, "/opt/trn_rl_repo")

from concourse import bacc, bass, masks, mybir, tile  # noqa: E402
from concourse.bass_utils import run_bass_kernel_spmd  # noqa: E402

B, P, L = 8, 196, 80
D1, D2, ATT = 2048, 300, 1024
NK = ATT // 128  # 8 att blocks
ND = D1 // 128  # 16 contraction blocks for a1
NJ = 3  # 300 -> 384 = 3*128 padded contraction blocks for a2
D2P = NJ * 128
GR = 8  # l-values per group (4 psum rows x 2 free slots)
NG = L // GR  # 10 groups
DVE_K = 5  # k-blocks 0..4 multiplied on DVE, 5..7 on GpSimd
F32 = mybir.dt.float32
F16 = mybir.dt.float16
BF16 = mybir.dt.bfloat16
AF = mybir.ActivationFunctionType

_CACHE: dict = {}


def _bcast(ap2d, n, pos):
    """Insert a stride-0 dim of size n into a 2D AP at free position pos."""
    dims = [list(d) for d in ap2d.ap]
    if pos == 0:
        new = [dims[0], [0, n], dims[1]]
    else:
        new = [dims[0], dims[1], [0, n]]
    return bass.AP(ap2d.tensor, ap2d.offset, new)


def _build():
    nc = bacc.Bacc(None, target_bir_lowering=False, debug=True)

    x1n_d = nc.dram_tensor("x1n", (P, D1), F32, kind="ExternalInput")
    x1t_d = nc.dram_tensor("x1t", (D1, P), BF16, kind="ExternalInput")
    x2t_d = nc.dram_tensor("x2t", (D2P, L), BF16, kind="ExternalInput")
    w1b_d = nc.dram_tensor("w1b", (NK, D1, 128), BF16, kind="ExternalInput")
    w2_d = nc.dram_tensor("w2p", (D2P, ATT), BF16, kind="ExternalInput")
    wv_d = nc.dram_tensor("wv", (128, NK), BF16, kind="ExternalInput")
    lab_d = nc.dram_tensor("lab", (L, D1), F32, kind="ExternalOutput")
    alp_d = nc.dram_tensor("alp", (L, P), F32, kind="ExternalOutput")

    with tile.TileContext(nc) as tc:
        with (
            tc.tile_pool(name="const", bufs=1) as cpool,
            tc.tile_pool(name="stream", bufs=2) as spool,
            tc.tile_pool(name="upool", bufs=4) as upool,
            tc.tile_pool(name="ps", bufs=1, space="PSUM") as pspool,
        ):
            ident = cpool.tile([128, 128], F32, tag="ident")
            masks.make_identity(nc, ident)

            wv_sb = cpool.tile([128, NK], BF16, tag="wv")
            nc.sync.dma_start(wv_sb, wv_d[:])

            x2t_sb = cpool.tile([128, NJ, L], BF16, tag="x2t")
            nc.sync.dma_start(x2t_sb, x2t_d[:].rearrange("(n p) l -> p n l", p=128))

            w2_sb = cpool.tile([128, NJ, ATT], BF16, tag="w2")
            nc.sync.dma_start(w2_sb, w2_d[:].rearrange("(n p) a -> p n a", p=128))

            x1t_sb = cpool.tile([128, ND, P], BF16, tag="x1t")
            nc.sync.dma_start(x1t_sb, x1t_d[:].rearrange("(n p) m -> p n m", p=128))

            a1sb = cpool.tile([128, NK, P], BF16, tag="a1sb")
            a2sb = cpool.tile([128, NK, L], BF16, tag="a2sb")
            a2f = cpool.tile([128, NK, L], F32, tag="a2f")

            # ---- stage A: a2T[k] = sum_j W2[j-block, k-block]^T @ x2T[j-block] ----
            for k in range(NK):
                ps_a2 = pspool.tile([128, L], F32, tag="ps_a2", bufs=1)
                for j in range(NJ):
                    nc.tensor.matmul(
                        ps_a2,
                        w2_sb[:, j, k * 128 : (k + 1) * 128],
                        x2t_sb[:, j, :],
                        start=(j == 0),
                        stop=(j == NJ - 1),
                    )
                nc.scalar.copy(a2sb[:, k, :], ps_a2)
                nc.scalar.copy(a2f[:, k, :], ps_a2)

            # ---- stage B: a1T[k] = sum_d W1[d-block, k-block]^T @ x1T[d-block] ----
            for k in range(NK):
                w1k = spool.tile([128, ND, 128], BF16, tag="w1k")
                nc.sync.dma_start(w1k, w1b_d[k].rearrange("(n p) m -> p n m", p=128))
                ps_a1 = pspool.tile([128, P], F32, tag="ps_a1", bufs=2)
                for d in range(ND):
                    nc.tensor.matmul(
                        ps_a1,
                        w1k[:, d, :],
                        x1t_sb[:, d, :],
                        start=(d == 0),
                        stop=(d == ND - 1),
                    )
                nc.scalar.copy(a1sb[:, k, :], ps_a1)

            # x1 natural layout, only needed by the label matmul at the end;
            # emitted here so the DMAs overlap the group loop.
            x1n0 = cpool.tile([128, D1], F32, tag="x1n0")
            nc.sync.dma_start(x1n0, x1n_d[0:128, :])
            x1n1 = cpool.tile([128, D1], F32, tag="x1n1")
            nc.sync.dma_start(x1n1[: P - 128, :], x1n_d[128:P, :])

            at0 = cpool.tile([128, L], F32, tag="at0")  # alphaT[0:128, :]
            at1 = cpool.tile([128, L], F32, tag="at1")  # alphaT[128:196, :]

            def emit_transposes(g, al2):
                # transpose the normalized alpha rows into at0/at1 columns:
                # l = GR*g + 2*irow + s lives at psum row 32*irow, slot s.
                # Emitted one group late so the PE never stalls waiting on the
                # softmax chain of the current group.
                for s in range(2):
                    pst0 = pspool.tile([128, 128], F32, tag="misc", bufs=2)
                    nc.tensor.transpose(pst0, al2[:, s, 0:128], ident)
                    nc.vector.tensor_copy(
                        at0[:, GR * g + s : GR * (g + 1) : 2], pst0[:, 0:128:32]
                    )
                    pst1 = pspool.tile([128, 128], F32, tag="misc", bufs=2)
                    nc.tensor.transpose(pst1[: P - 128, :], al2[:, s, 128:P], ident)
                    nc.vector.tensor_copy(
                        at1[: P - 128, GR * g + s : GR * (g + 1) : 2],
                        pst1[: P - 128, 0:128:32],
                    )

            # ---- stage C: per group of GR=8 l-values ----
            # 3 manually-rotated score banks, each initialized once; later
            # groups inherit finite garbage in the unused rows, which softmax
            # renders harmless (exp(x - max) <= 1, sum >= 1).
            banks = []
            for bi in range(3):
                bk = pspool.tile([128, 512], F32, tag=f"score{bi}", bufs=1,
                                 name=f"bank{bi}")
                nc.vector.memset(bk, 0.0)
                banks.append(bk)
            for g in range(NG):
                # separate tiles for the DVE-written (k<4) and GpSimd-written
                # (k>=4) halves: cross-engine writes to one tile serialize in
                # the scheduler's dependency tracking.
                up_g = upool.tile([128, 4, GR, P], BF16, tag="up_g")
                for k in range(4, NK):
                    nc.gpsimd.tensor_tensor(
                        up_g[:, k - 4, :, :],
                        _bcast(a1sb[:, k, :], GR, 0),
                        _bcast(a2sb[:, k, GR * g : GR * (g + 1)], P, 1),
                        op=mybir.AluOpType.mult,
                    )
                up_d = upool.tile([128, 4, GR, P], BF16, tag="up_d")
                for k in range(4):
                    a1k = a1sb[:, k, :]
                    for i in range(GR):
                        nc.vector.tensor_scalar_mul(
                            up_d[:, k, i, :], a1k,
                            a2f[:, k, GR * g + i : GR * g + i + 1],
                        )
                for kk in range(0, 4, 2):
                    nc.scalar.activation(
                        up_d[:, kk : kk + 2, :, :], up_d[:, kk : kk + 2, :, :],
                        AF.Tanh,
                    )
                    nc.scalar.activation(
                        up_g[:, kk : kk + 2, :, :], up_g[:, kk : kk + 2, :, :],
                        AF.Tanh,
                    )

                bank = banks[g % 3]
                for k in range(NK):
                    usrc = up_d if k < 4 else up_g
                    for i in range(4):
                        nc.tensor.matmul(
                            bank[32 * i : 32 * i + 1, 0 : 2 * P],
                            wv_sb[:, k : k + 1],
                            usrc[:, k % 4, 2 * i : 2 * i + 2, :],
                            start=(k == 0),
                            stop=(k == NK - 1),
                            skip_group_check=True,
                            tile_position=(0, 32 * i),
                        )

                # softmax over p per (row, slot); rows other than {0,32,64,96}
                # hold garbage and are never read downstream.
                if g > 0:
                    emit_transposes(g - 1, prev_al2)
                sc3 = bank[:, 0 : 2 * P].rearrange("p (s q) -> p s q", q=P)
                mx2 = spool.tile([128, 2], F32, tag="mx2")
                nc.vector.tensor_reduce(
                    mx2, sc3, axis=mybir.AxisListType.X, op=mybir.AluOpType.max,
                    negate=True,
                )
                ex2 = spool.tile([128, 2, P], F32, tag="ex2")
                sm2 = spool.tile([128, 2], F32, tag="sm2")
                for s in range(2):
                    nc.scalar.activation(
                        ex2[:, s, :],
                        bank[:, s * P : (s + 1) * P],
                        AF.Exp,
                        bias=mx2[:, s : s + 1],
                        scale=1.0,
                        accum_out=sm2[:, s : s + 1],
                    )
                rs2 = spool.tile([128, 2], F32, tag="rs2")
                nc.vector.reciprocal(rs2, sm2)
                al2 = spool.tile([128, 2, P], F32, tag="al2")
                for s in range(2):
                    nc.vector.tensor_scalar_mul(
                        al2[:, s, :], ex2[:, s, :], rs2[:, s : s + 1]
                    )
                nc.sync.dma_start(alp_d[GR * g : GR * (g + 1), :], al2[0:128:32, :, :])
                prev_al2 = al2


            emit_transposes(NG - 1, prev_al2)

            # ---- label = alphaT^T @ x1 ----
            for n in range(D1 // 512):
                ps_lab = pspool.tile([L, 512], F32, tag="misc", bufs=2)
                nc.tensor.matmul(
                    ps_lab,
                    at0,
                    x1n0[:, n * 512 : (n + 1) * 512],
                    start=True,
                    stop=False,
                )
                nc.tensor.matmul(
                    ps_lab,
                    at1[: P - 128, :],
                    x1n1[: P - 128, n * 512 : (n + 1) * 512],
                    start=False,
                    stop=True,
                )
                lab_sb = spool.tile([L, 512], F32, tag="lab_sb")
                nc.vector.tensor_copy(lab_sb, ps_lab)
                nc.sync.dma_start(lab_d[:, n * 512 : (n + 1) * 512], lab_sb)

    nc.compile()
    return nc


def get_nc():
    if "nc" not in _CACHE:
        _CACHE["nc"] = _build()
    return _CACHE["nc"]


def _host_prep(x1, x2, W1, W2, Wh, bh, Wt, bt):
    """Build the 8 per-core input maps (host-side sharding / relayout)."""
    x1 = np.asarray(x1, dtype=np.float32)
    x2 = np.asarray(x2, dtype=np.float32)
    W1 = np.asarray(W1, dtype=np.float32)
    W2 = np.asarray(W2, dtype=np.float32)
    w = (np.asarray(Wh, np.float32) @ np.asarray(Wt, np.float32)).astype(np.float32)

    w1b = np.ascontiguousarray(
        W1.reshape(D1, NK, 128).transpose(1, 0, 2)
    ).astype(mybir.dt.np(BF16))  # (NK, D1, 128)
    w2p = np.zeros((D2P, ATT), mybir.dt.np(BF16))
    w2p[:D2, :] = W2.astype(mybir.dt.np(BF16))
    wv = np.ascontiguousarray(w.reshape(NK, 128).T).astype(mybir.dt.np(BF16))  # (128, NK)

    in_maps = []
    for b in range(B):
        x2t = np.zeros((D2P, L), mybir.dt.np(BF16))
        x2t[:D2, :] = x2[b].T.astype(mybir.dt.np(BF16))
        in_maps.append(
            {
                "x1n": x1[b],
                "x1t": np.ascontiguousarray(x1[b].T).astype(mybir.dt.np(BF16)),
                "x2t": x2t,
                "w1b": w1b,
                "w2p": w2p,
                "wv": wv,
            }
        )
    return in_maps


def kernel(x1, x2, W1, W2, Wh, bh, Wt, bt):
    nc = get_nc()
    in_maps = _host_prep(x1, x2, W1, W2, Wh, bh, Wt, bt)
    res = run_bass_kernel_spmd(nc, in_maps, core_ids=list(range(B)))
    label = np.stack([res.results[b]["lab"] for b in range(B)])
    alpha = np.stack([res.results[b]["alp"] for b in range(B)])
    return label, alpha


# revision 16
# speedup vs baseline: 1.2001x; 1.2001x over previous
"""Trainium2 Bass kernel for LowRankBilinearAttention.

Reference computation (per batch b):
    a1 = x1 @ W1                  # (P, ATT)
    a2 = x2 @ W2                  # (L, ATT)
    u  = tanh(a1[p,:] * a2[l,:])  # (L, P, ATT)
    w  = Wh @ Wt                  # (ATT,)   [folded on host]
    score[l,p] = u . w            # + (bh@Wt + bt) -- a scalar, cancels in softmax
    alpha = softmax(score, axis=p)
    label = alpha @ x1            # (L, DIM1)

Sharding: pure data-parallel over batch B=8 -> one batch per NeuronCore,
no collectives.

Device layout (per core), fp16 datapath with fp32 accumulation:
    ATT on partitions (8 blocks of 128) for the u stage.  Per group of
    GR=8 l-values:
      - DVE/GpSimd tensor_tensor with stride-0 broadcast APs:
        up[a, k, l, p] = a1T[a,p] * a2T[a,l]      (fp16, one instr per k)
      - ACT tanh in-place over the whole (128, 12544) chunk
      - PE: M=1 fp16 matmuls (lhsT = w column) with N=392 (two l per psum
        row), accumulating over k into one PSUM bank at rows {0,32,64,96}
        (legal output base partitions via tile_position).
    Softmax is per-partition per-slot (ACT exp with per-partition bias and
    accum_out), so the scattered row layout is fine; alpha rows are gathered
    by partition-strided DMA; alpha is normalized before the PE transpose so
    the label matmul needs no rescale.
"""

import sys

import numpy as np

if "/opt/trn_rl_repo" not in sys.path:
    sys.path.insert(0, "/opt/trn_rl_repo")

from concourse import bacc, bass, masks, mybir, tile  # noqa: E402
from concourse.bass_utils import run_bass_kernel_spmd  # noqa: E402

B, P, L = 8, 196, 80
D1, D2, ATT = 2048, 300, 1024
NK = ATT // 128  # 8 att blocks
ND = D1 // 128  # 16 contraction blocks for a1
NJ = 3  # 300 -> 384 = 3*128 padded contraction blocks for a2
D2P = NJ * 128
GR = 8  # l-values per group (4 psum rows x 2 free slots)
NG = L // GR  # 10 groups
DVE_K = 5  # k-blocks 0..4 multiplied on DVE, 5..7 on GpSimd
F32 = mybir.dt.float32
F16 = mybir.dt.float16
BF16 = mybir.dt.bfloat16
AF = mybir.ActivationFunctionType

_CACHE: dict = {}


def _bcast(ap2d, n, pos):
    """Insert a stride-0 dim of size n into a 2D AP at free position pos."""
    dims = [list(d) for d in ap2d.ap]
    if pos == 0:
        new = [dims[0], [0, n], dims[1]]
    else:
        new = [dims[0], dims[1], [0, n]]
    return bass.AP(ap2d.tensor, ap2d.offset, new)


def _build():
    nc = bacc.Bacc(None, target_bir_lowering=False, debug=True)

    x1n_d = nc.dram_tensor("x1n", (P, D1), F32, kind="ExternalInput")
    x1t_d = nc.dram_tensor("x1t", (D1, P), BF16, kind="ExternalInput")
    x2t_d = nc.dram_tensor("x2t", (D2P, L), BF16, kind="ExternalInput")
    w1b_d = nc.dram_tensor("w1b", (NK, D1, 128), BF16, kind="ExternalInput")
    w2_d = nc.dram_tensor("w2p", (D2P, ATT), BF16, kind="ExternalInput")
    wv_d = nc.dram_tensor("wv", (128, NK), BF16, kind="ExternalInput")
    lab_d = nc.dram_tensor("lab", (L, D1), F32, kind="ExternalOutput")
    alp_d = nc.dram_tensor("alp", (L, P), F32, kind="ExternalOutput")

    with tile.TileContext(nc) as tc:
        with (
            tc.tile_pool(name="const", bufs=1) as cpool,
            tc.tile_pool(name="stream", bufs=2) as spool,
            tc.tile_pool(name="upool", bufs=4) as upool,
            tc.tile_pool(name="ps", bufs=1, space="PSUM") as pspool,
        ):
            ident = cpool.tile([128, 128], F32, tag="ident")
            masks.make_identity(nc, ident)

            wv_sb = cpool.tile([128, NK], BF16, tag="wv")
            nc.sync.dma_start(wv_sb, wv_d[:])

            x2t_sb = cpool.tile([128, NJ, L], BF16, tag="x2t")
            nc.sync.dma_start(x2t_sb, x2t_d[:].rearrange("(n p) l -> p n l", p=128))

            w2_sb = cpool.tile([128, NJ, ATT], BF16, tag="w2")
            nc.sync.dma_start(w2_sb, w2_d[:].rearrange("(n p) a -> p n a", p=128))

            x1t_sb = cpool.tile([128, ND, P], BF16, tag="x1t")
            nc.sync.dma_start(x1t_sb, x1t_d[:].rearrange("(n p) m -> p n m", p=128))

            a1sb = cpool.tile([128, NK, P], BF16, tag="a1sb")
            a2sb = cpool.tile([128, NK, L], BF16, tag="a2sb")
            a2f = cpool.tile([128, NK, L], F32, tag="a2f")

            # ---- stage A: a2T[k] = sum_j W2[j-block, k-block]^T @ x2T[j-block] ----
            for k in range(NK):
                ps_a2 = pspool.tile([128, L], F32, tag="ps_a2", bufs=1)
                for j in range(NJ):
                    nc.tensor.matmul(
                        ps_a2,
                        w2_sb[:, j, k * 128 : (k + 1) * 128],
                        x2t_sb[:, j, :],
                        start=(j == 0),
                        stop=(j == NJ - 1),
                    )
                nc.scalar.copy(a2sb[:, k, :], ps_a2)
                nc.scalar.copy(a2f[:, k, :], ps_a2)

            # ---- stage B: a1T[k] = sum_d W1[d-block, k-block]^T @ x1T[d-block] ----
            for k in range(NK):
                w1k = spool.tile([128, ND, 128], BF16, tag="w1k")
                nc.sync.dma_start(w1k, w1b_d[k].rearrange("(n p) m -> p n m", p=128))
                ps_a1 = pspool.tile([128, P], F32, tag="ps_a1", bufs=2)
                for d in range(ND):
                    nc.tensor.matmul(
                        ps_a1,
                        w1k[:, d, :],
                        x1t_sb[:, d, :],
                        start=(d == 0),
                        stop=(d == ND - 1),
                    )
                nc.scalar.copy(a1sb[:, k, :], ps_a1)

            # x1 natural layout, only needed by the label matmul at the end;
            # emitted here so the DMAs overlap the group loop.
            x1n0 = cpool.tile([128, D1], F32, tag="x1n0")
            nc.sync.dma_start(x1n0, x1n_d[0:128, :])
            x1n1 = cpool.tile([128, D1], F32, tag="x1n1")
            nc.sync.dma_start(x1n1[: P - 128, :], x1n_d[128:P, :])

            at0 = cpool.tile([128, L], F32, tag="at0")  # alphaT[0:128, :]
            at1 = cpool.tile([128, L], F32, tag="at1")  # alphaT[128:196, :]

            def emit_transposes(g, al2):
                # transpose the normalized alpha rows into at0/at1 columns:
                # l = GR*g + 2*irow + s lives at psum row 32*irow, slot s.
                # Emitted one group late so the PE never stalls waiting on the
                # softmax chain of the current group.
                for s in range(2):
                    pst0 = pspool.tile([128, 128], F32, tag="misc", bufs=2)
                    nc.tensor.transpose(pst0, al2[:, s, 0:128], ident)
                    nc.vector.tensor_copy(
                        at0[:, GR * g + s : GR * (g + 1) : 2], pst0[:, 0:128:32]
                    )
                    pst1 = pspool.tile([128, 128], F32, tag="misc", bufs=2)
                    nc.tensor.transpose(pst1[: P - 128, :], al2[:, s, 128:P], ident)
                    nc.vector.tensor_copy(
                        at1[: P - 128, GR * g + s : GR * (g + 1) : 2],
                        pst1[: P - 128, 0:128:32],
                    )

            # ---- stage C: per group of GR=8 l-values ----
            # 3 manually-rotated score banks, each initialized once; later
            # groups inherit finite garbage in the unused rows, which softmax
            # renders harmless (exp(x - max) <= 1, sum >= 1).
            banks = []
            for bi in range(3):
                bk = pspool.tile([128, 512], F32, tag=f"score{bi}", bufs=1,
                                 name=f"bank{bi}")
                nc.vector.memset(bk, 0.0)
                banks.append(bk)
            for g in range(NG):
                # mul stage: DVE tensor_scalar for k<6; for k in {6,7} the
                # multiply rides the ACT tanh as a per-partition scale
                # (GpSimd 2-input ops contend for the DVE SBUF ports, so it
                # gets no elementwise work).  Separate tiles per writer engine
                # keep the scheduler from serializing cross-engine writes.
                up_d = upool.tile([128, 6, GR, P], BF16, tag="up_d")
                for k in range(6):
                    a1k = a1sb[:, k, :]
                    for i in range(GR):
                        nc.vector.tensor_scalar_mul(
                            up_d[:, k, i, :], a1k,
                            a2f[:, k, GR * g + i : GR * g + i + 1],
                        )
                up_a = upool.tile([128, 2, GR, P], BF16, tag="up_a")
                for k in range(6, NK):
                    a1k = a1sb[:, k, :]
                    for i in range(GR):
                        nc.scalar.activation(
                            up_a[:, k - 6, i, :], a1k, AF.Tanh,
                            scale=a2f[:, k, GR * g + i : GR * g + i + 1],
                        )
                for kk in range(0, 6, 2):
                    nc.scalar.activation(
                        up_d[:, kk : kk + 2, :, :], up_d[:, kk : kk + 2, :, :],
                        AF.Tanh,
                    )

                bank = banks[g % 3]
                for k in range(NK):
                    usrc = up_d if k < 6 else up_a
                    for i in range(4):
                        nc.tensor.matmul(
                            bank[32 * i : 32 * i + 1, 0 : 2 * P],
                            wv_sb[:, k : k + 1],
                            usrc[:, k if k < 6 else k - 6, 2 * i : 2 * i + 2, :],
                            start=(k == 0),
                            stop=(k == NK - 1),
                            skip_group_check=True,
                            tile_position=(0, 32 * i),
                        )

                # softmax over p per (row, slot); rows other than {0,32,64,96}
                # hold garbage and are never read downstream.
                if g > 0:
                    emit_transposes(g - 1, prev_al2)
                sc3 = bank[:, 0 : 2 * P].rearrange("p (s q) -> p s q", q=P)
                mx2 = spool.tile([128, 2], F32, tag="mx2")
                nc.vector.tensor_reduce(
                    mx2, sc3, axis=mybir.AxisListType.X, op=mybir.AluOpType.max,
                    negate=True,
                )
                ex2 = spool.tile([128, 2, P], F32, tag="ex2")
                sm2 = spool.tile([128, 2], F32, tag="sm2")
                for s in range(2):
                    nc.scalar.activation(
                        ex2[:, s, :],
                        bank[:, s * P : (s + 1) * P],
                        AF.Exp,
                        bias=mx2[:, s : s + 1],
                        scale=1.0,
                        accum_out=sm2[:, s : s + 1],
                    )
                rs2 = spool.tile([128, 2], F32, tag="rs2")
                nc.vector.reciprocal(rs2, sm2)
                al2 = spool.tile([128, 2, P], F32, tag="al2")
                for s in range(2):
                    nc.vector.tensor_scalar_mul(
                        al2[:, s, :], ex2[:, s, :], rs2[:, s : s + 1]
                    )
                nc.sync.dma_start(alp_d[GR * g : GR * (g + 1), :], al2[0:128:32, :, :])
                prev_al2 = al2


            emit_transposes(NG - 1, prev_al2)

            # ---- label = alphaT^T @ x1 ----
            for n in range(D1 // 512):
                ps_lab = pspool.tile([L, 512], F32, tag="misc", bufs=2)
                nc.tensor.matmul(
                    ps_lab,
                    at0,
                    x1n0[:, n * 512 : (n + 1) * 512],
                    start=True,
                    stop=False,
                )
                nc.tensor.matmul(
                    ps_lab,
                    at1[: P - 128, :],
                    x1n1[: P - 128, n * 512 : (n + 1) * 512],
                    start=False,
                    stop=True,
                )
                lab_sb = spool.tile([L, 512], F32, tag="lab_sb")
                nc.vector.tensor_copy(lab_sb, ps_lab)
                nc.sync.dma_start(lab_d[:, n * 512 : (n + 1) * 512], lab_sb)

    nc.compile()
    return nc


def get_nc():
    if "nc" not in _CACHE:
        _CACHE["nc"] = _build()
    return _CACHE["nc"]


def _host_prep(x1, x2, W1, W2, Wh, bh, Wt, bt):
    """Build the 8 per-core input maps (host-side sharding / relayout)."""
    x1 = np.asarray(x1, dtype=np.float32)
    x2 = np.asarray(x2, dtype=np.float32)
    W1 = np.asarray(W1, dtype=np.float32)
    W2 = np.asarray(W2, dtype=np.float32)
    w = (np.asarray(Wh, np.float32) @ np.asarray(Wt, np.float32)).astype(np.float32)

    w1b = np.ascontiguousarray(
        W1.reshape(D1, NK, 128).transpose(1, 0, 2)
    ).astype(mybir.dt.np(BF16))  # (NK, D1, 128)
    w2p = np.zeros((D2P, ATT), mybir.dt.np(BF16))
    w2p[:D2, :] = W2.astype(mybir.dt.np(BF16))
    wv = np.ascontiguousarray(w.reshape(NK, 128).T).astype(mybir.dt.np(BF16))  # (128, NK)

    in_maps = []
    for b in range(B):
        x2t = np.zeros((D2P, L), mybir.dt.np(BF16))
        x2t[:D2, :] = x2[b].T.astype(mybir.dt.np(BF16))
        in_maps.append(
            {
                "x1n": x1[b],
                "x1t": np.ascontiguousarray(x1[b].T).astype(mybir.dt.np(BF16)),
                "x2t": x2t,
                "w1b": w1b,
                "w2p": w2p,
                "wv": wv,
            }
        )
    return in_maps


def kernel(x1, x2, W1, W2, Wh, bh, Wt, bt):
    nc = get_nc()
    in_maps = _host_prep(x1, x2, W1, W2, Wh, bh, Wt, bt)
    res = run_bass_kernel_spmd(nc, in_maps, core_ids=list(range(B)))
    label = np.stack([res.results[b]["lab"] for b in range(B)])
    alpha = np.stack([res.results[b]["alp"] for b in range(B)])
    return label, alpha


# revision 17
# speedup vs baseline: 1.2442x; 1.0367x over previous
"""Trainium2 Bass kernel for LowRankBilinearAttention.

Reference computation (per batch b):
    a1 = x1 @ W1                  # (P, ATT)
    a2 = x2 @ W2                  # (L, ATT)
    u  = tanh(a1[p,:] * a2[l,:])  # (L, P, ATT)
    w  = Wh @ Wt                  # (ATT,)   [folded on host]
    score[l,p] = u . w            # + (bh@Wt + bt) -- a scalar, cancels in softmax
    alpha = softmax(score, axis=p)
    label = alpha @ x1            # (L, DIM1)

Sharding: pure data-parallel over batch B=8 -> one batch per NeuronCore,
no collectives.

Device layout (per core), fp16 datapath with fp32 accumulation:
    ATT on partitions (8 blocks of 128) for the u stage.  Per group of
    GR=8 l-values:
      - DVE/GpSimd tensor_tensor with stride-0 broadcast APs:
        up[a, k, l, p] = a1T[a,p] * a2T[a,l]      (fp16, one instr per k)
      - ACT tanh in-place over the whole (128, 12544) chunk
      - PE: M=1 fp16 matmuls (lhsT = w column) with N=392 (two l per psum
        row), accumulating over k into one PSUM bank at rows {0,32,64,96}
        (legal output base partitions via tile_position).
    Softmax is per-partition per-slot (ACT exp with per-partition bias and
    accum_out), so the scattered row layout is fine; alpha rows are gathered
    by partition-strided DMA; alpha is normalized before the PE transpose so
    the label matmul needs no rescale.
"""

import sys

import numpy as np

if "/opt/trn_rl_repo" not in sys.path:
    sys.path.insert(0, "/opt/trn_rl_repo")

from concourse import bacc, bass, masks, mybir, tile  # noqa: E402
from concourse.bass_utils import run_bass_kernel_spmd  # noqa: E402

B, P, L = 8, 196, 80
D1, D2, ATT = 2048, 300, 1024
NK = ATT // 128  # 8 att blocks
ND = D1 // 128  # 16 contraction blocks for a1
NJ = 3  # 300 -> 384 = 3*128 padded contraction blocks for a2
D2P = NJ * 128
GR = 8  # l-values per group (4 psum rows x 2 free slots)
NG = L // GR  # 10 groups
DVE_K = 5  # k-blocks 0..4 multiplied on DVE, 5..7 on GpSimd
F32 = mybir.dt.float32
F16 = mybir.dt.float16
BF16 = mybir.dt.bfloat16
AF = mybir.ActivationFunctionType

_CACHE: dict = {}


def _bcast(ap2d, n, pos):
    """Insert a stride-0 dim of size n into a 2D AP at free position pos."""
    dims = [list(d) for d in ap2d.ap]
    if pos == 0:
        new = [dims[0], [0, n], dims[1]]
    else:
        new = [dims[0], dims[1], [0, n]]
    return bass.AP(ap2d.tensor, ap2d.offset, new)


def _build():
    nc = bacc.Bacc(None, target_bir_lowering=False, debug=True)

    x1n_d = nc.dram_tensor("x1n", (P, D1), F32, kind="ExternalInput")
    x1t_d = nc.dram_tensor("x1t", (D1, P), BF16, kind="ExternalInput")
    x2t_d = nc.dram_tensor("x2t", (D2P, L), BF16, kind="ExternalInput")
    w1b_d = nc.dram_tensor("w1b", (NK, D1, 128), BF16, kind="ExternalInput")
    w2_d = nc.dram_tensor("w2p", (D2P, ATT), BF16, kind="ExternalInput")
    wv_d = nc.dram_tensor("wv", (128, NK), BF16, kind="ExternalInput")
    lab_d = nc.dram_tensor("lab", (L, D1), F32, kind="ExternalOutput")
    alp_d = nc.dram_tensor("alp", (L, P), F32, kind="ExternalOutput")

    with tile.TileContext(nc) as tc:
        with (
            tc.tile_pool(name="const", bufs=1) as cpool,
            tc.tile_pool(name="stream", bufs=2) as spool,
            tc.tile_pool(name="upool", bufs=4) as upool,
            tc.tile_pool(name="ps", bufs=1, space="PSUM") as pspool,
        ):
            ident = cpool.tile([128, 128], F32, tag="ident")
            masks.make_identity(nc, ident)

            wv_sb = cpool.tile([128, NK], BF16, tag="wv")
            nc.sync.dma_start(wv_sb, wv_d[:])

            x2t_sb = cpool.tile([128, NJ, L], BF16, tag="x2t")
            nc.sync.dma_start(x2t_sb, x2t_d[:].rearrange("(n p) l -> p n l", p=128))

            w2_sb = cpool.tile([128, NJ, ATT], BF16, tag="w2")
            nc.sync.dma_start(w2_sb, w2_d[:].rearrange("(n p) a -> p n a", p=128))

            x1t_sb = cpool.tile([128, ND, P], BF16, tag="x1t")
            nc.sync.dma_start(x1t_sb, x1t_d[:].rearrange("(n p) m -> p n m", p=128))

            a1sb = cpool.tile([128, NK, P], BF16, tag="a1sb")
            a2sb = cpool.tile([128, NK, L], BF16, tag="a2sb")
            a2f = cpool.tile([128, NK, L], F32, tag="a2f")

            # ---- stage A: a2T[k] = sum_j W2[j-block, k-block]^T @ x2T[j-block] ----
            for k in range(NK):
                ps_a2 = pspool.tile([128, L], F32, tag="ps_a2", bufs=1)
                for j in range(NJ):
                    nc.tensor.matmul(
                        ps_a2,
                        w2_sb[:, j, k * 128 : (k + 1) * 128],
                        x2t_sb[:, j, :],
                        start=(j == 0),
                        stop=(j == NJ - 1),
                    )
                nc.scalar.copy(a2sb[:, k, :], ps_a2)
                nc.scalar.copy(a2f[:, k, :], ps_a2)

            # ---- stage B: a1T[k] = sum_d W1[d-block, k-block]^T @ x1T[d-block] ----
            for k in range(NK):
                w1k = spool.tile([128, ND, 128], BF16, tag="w1k")
                nc.sync.dma_start(w1k, w1b_d[k].rearrange("(n p) m -> p n m", p=128))
                ps_a1 = pspool.tile([128, P], F32, tag="ps_a1", bufs=2)
                for d in range(ND):
                    nc.tensor.matmul(
                        ps_a1,
                        w1k[:, d, :],
                        x1t_sb[:, d, :],
                        start=(d == 0),
                        stop=(d == ND - 1),
                    )
                nc.scalar.copy(a1sb[:, k, :], ps_a1)

            # x1 natural layout, only needed by the label matmul at the end;
            # emitted here so the DMAs overlap the group loop.
            x1n0 = cpool.tile([128, D1], F32, tag="x1n0")
            nc.sync.dma_start(x1n0, x1n_d[0:128, :])
            x1n1 = cpool.tile([128, D1], F32, tag="x1n1")
            nc.sync.dma_start(x1n1[: P - 128, :], x1n_d[128:P, :])

            at0 = cpool.tile([128, L], F32, tag="at0")  # alphaT[0:128, :]
            at1 = cpool.tile([128, L], F32, tag="at1")  # alphaT[128:196, :]

            def emit_transposes(g, al2):
                # transpose the normalized alpha rows into at0/at1 columns:
                # l = GR*g + 2*irow + s lives at psum row 32*irow, slot s.
                # Emitted one group late so the PE never stalls waiting on the
                # softmax chain of the current group.
                for s in range(2):
                    pst0 = pspool.tile([128, 128], F32, tag="misc", bufs=2)
                    nc.tensor.transpose(pst0, al2[:, s, 0:128], ident)
                    nc.vector.tensor_copy(
                        at0[:, GR * g + s : GR * (g + 1) : 2], pst0[:, 0:128:32]
                    )
                    pst1 = pspool.tile([128, 128], F32, tag="misc", bufs=2)
                    nc.tensor.transpose(pst1[: P - 128, :], al2[:, s, 128:P], ident)
                    nc.vector.tensor_copy(
                        at1[: P - 128, GR * g + s : GR * (g + 1) : 2],
                        pst1[: P - 128, 0:128:32],
                    )

            # ---- stage C: per group of GR=8 l-values ----
            # 3 manually-rotated score banks, each initialized once; later
            # groups inherit finite garbage in the unused rows, which softmax
            # renders harmless (exp(x - max) <= 1, sum >= 1).
            banks = []
            for bi in range(3):
                bk = pspool.tile([128, 512], F32, tag=f"score{bi}", bufs=1,
                                 name=f"bank{bi}")
                nc.vector.memset(bk, 0.0)
                banks.append(bk)
            for g in range(NG):
                # mul stage, balanced so DVE and ACT finish together:
                # DVE tensor_scalar (263ns/chunk) for k<6 and the back half of
                # k=7; ACT fused scale+tanh (551ns/chunk) for k=6 and the
                # front half of k=7.  Separate tiles per writer engine.
                up_d = upool.tile([128, 6, GR, P], BF16, tag="up_d")
                for k in range(6):
                    a1k = a1sb[:, k, :]
                    for i in range(GR):
                        nc.vector.tensor_scalar_mul(
                            up_d[:, k, i, :], a1k,
                            a2f[:, k, GR * g + i : GR * g + i + 1],
                        )
                up7d = upool.tile([128, GR // 2, P], BF16, tag="up7d")
                for i in range(GR // 2, GR):
                    nc.vector.tensor_scalar_mul(
                        up7d[:, i - GR // 2, :], a1sb[:, 7, :],
                        a2f[:, 7, GR * g + i : GR * g + i + 1],
                    )
                up_a = upool.tile([128, GR, P], BF16, tag="up_a")
                for i in range(GR):
                    nc.scalar.activation(
                        up_a[:, i, :], a1sb[:, 6, :], AF.Tanh,
                        scale=a2f[:, 6, GR * g + i : GR * g + i + 1],
                    )
                up7a = upool.tile([128, GR // 2, P], BF16, tag="up7a")
                for i in range(GR // 2):
                    nc.scalar.activation(
                        up7a[:, i, :], a1sb[:, 7, :], AF.Tanh,
                        scale=a2f[:, 7, GR * g + i : GR * g + i + 1],
                    )
                for kk in range(0, 6, 2):
                    nc.scalar.activation(
                        up_d[:, kk : kk + 2, :, :], up_d[:, kk : kk + 2, :, :],
                        AF.Tanh,
                    )
                nc.scalar.activation(up7d, up7d, AF.Tanh)

                bank = banks[g % 3]
                for k in range(NK):
                    for i in range(4):
                        if k < 6:
                            rhs = up_d[:, k, 2 * i : 2 * i + 2, :]
                        elif k == 6:
                            rhs = up_a[:, 2 * i : 2 * i + 2, :]
                        elif i < 2:
                            rhs = up7a[:, 2 * i : 2 * i + 2, :]
                        else:
                            rhs = up7d[:, 2 * i - 4 : 2 * i - 2, :]
                        nc.tensor.matmul(
                            bank[32 * i : 32 * i + 1, 0 : 2 * P],
                            wv_sb[:, k : k + 1],
                            rhs,
                            start=(k == 0),
                            stop=(k == NK - 1),
                            skip_group_check=True,
                            tile_position=(0, 32 * i),
                        )

                # softmax over p per (row, slot); rows other than {0,32,64,96}
                # hold garbage and are never read downstream.
                if g > 0:
                    emit_transposes(g - 1, prev_al2)
                sc3 = bank[:, 0 : 2 * P].rearrange("p (s q) -> p s q", q=P)
                mx2 = spool.tile([128, 2], F32, tag="mx2")
                nc.vector.tensor_reduce(
                    mx2, sc3, axis=mybir.AxisListType.X, op=mybir.AluOpType.max,
                    negate=True,
                )
                ex2 = spool.tile([128, 2, P], F32, tag="ex2")
                sm2 = spool.tile([128, 2], F32, tag="sm2")
                for s in range(2):
                    nc.scalar.activation(
                        ex2[:, s, :],
                        bank[:, s * P : (s + 1) * P],
                        AF.Exp,
                        bias=mx2[:, s : s + 1],
                        scale=1.0,
                        accum_out=sm2[:, s : s + 1],
                    )
                rs2 = spool.tile([128, 2], F32, tag="rs2")
                nc.vector.reciprocal(rs2, sm2)
                al2 = spool.tile([128, 2, P], F32, tag="al2")
                for s in range(2):
                    nc.vector.tensor_scalar_mul(
                        al2[:, s, :], ex2[:, s, :], rs2[:, s : s + 1]
                    )
                nc.sync.dma_start(alp_d[GR * g : GR * (g + 1), :], al2[0:128:32, :, :])
                prev_al2 = al2


            emit_transposes(NG - 1, prev_al2)

            # ---- label = alphaT^T @ x1 ----
            for n in range(D1 // 512):
                ps_lab = pspool.tile([L, 512], F32, tag="misc", bufs=2)
                nc.tensor.matmul(
                    ps_lab,
                    at0,
                    x1n0[:, n * 512 : (n + 1) * 512],
                    start=True,
                    stop=False,
                )
                nc.tensor.matmul(
                    ps_lab,
                    at1[: P - 128, :],
                    x1n1[: P - 128, n * 512 : (n + 1) * 512],
                    start=False,
                    stop=True,
                )
                lab_sb = spool.tile([L, 512], F32, tag="lab_sb")
                nc.vector.tensor_copy(lab_sb, ps_lab)
                nc.sync.dma_start(lab_d[:, n * 512 : (n + 1) * 512], lab_sb)

    nc.compile()
    return nc


def get_nc():
    if "nc" not in _CACHE:
        _CACHE["nc"] = _build()
    return _CACHE["nc"]


def _host_prep(x1, x2, W1, W2, Wh, bh, Wt, bt):
    """Build the 8 per-core input maps (host-side sharding / relayout)."""
    x1 = np.asarray(x1, dtype=np.float32)
    x2 = np.asarray(x2, dtype=np.float32)
    W1 = np.asarray(W1, dtype=np.float32)
    W2 = np.asarray(W2, dtype=np.float32)
    w = (np.asarray(Wh, np.float32) @ np.asarray(Wt, np.float32)).astype(np.float32)

    w1b = np.ascontiguousarray(
        W1.reshape(D1, NK, 128).transpose(1, 0, 2)
    ).astype(mybir.dt.np(BF16))  # (NK, D1, 128)
    w2p = np.zeros((D2P, ATT), mybir.dt.np(BF16))
    w2p[:D2, :] = W2.astype(mybir.dt.np(BF16))
    wv = np.ascontiguousarray(w.reshape(NK, 128).T).astype(mybir.dt.np(BF16))  # (128, NK)

    in_maps = []
    for b in range(B):
        x2t = np.zeros((D2P, L), mybir.dt.np(BF16))
        x2t[:D2, :] = x2[b].T.astype(mybir.dt.np(BF16))
        in_maps.append(
            {
                "x1n": x1[b],
                "x1t": np.ascontiguousarray(x1[b].T).astype(mybir.dt.np(BF16)),
                "x2t": x2t,
                "w1b": w1b,
                "w2p": w2p,
                "wv": wv,
            }
        )
    return in_maps


def kernel(x1, x2, W1, W2, Wh, bh, Wt, bt):
    nc = get_nc()
    in_maps = _host_prep(x1, x2, W1, W2, Wh, bh, Wt, bt)
    res = run_bass_kernel_spmd(nc, in_maps, core_ids=list(range(B)))
    label = np.stack([res.results[b]["lab"] for b in range(B)])
    alpha = np.stack([res.results[b]["alp"] for b in range(B)])
    return label, alpha
